# revision 15
# baseline (speedup 1.0000x reference)
"""Bass/Trainium2 kernel for nn_MultiHeadedAttention (GQA + RoPE + causal attention).

Sharding: 8 cores = 2 batch groups x 4 head-groups.
Core c: batch b=c//4, head group j=c%4 (q heads 4j..4j+3, kv head j).

Output projection is column-sharded after AllGathers of ctx^T.
Attention runs head-major; heads 0-2 each fire one full AllGather as
they finish (at 25%/50%/75% of the phase) and the last head fires two
half-gathers (at 87.5%/100%), so the serial collective chain hides
under attention compute and only the final ~1MB trails the phase.
Phase 3 runs its accumulation chains h-major with 4 PSUM banks so all
h<3 matmuls proceed while the last head's AllGathers land.

Compute is bf16 on the TensorEngine (fp32 PSUM accumulation).

Other structure:
- startup: x tiles on the Sync DMA queue, weights/tables on the Scalar
  DMA queue, per-h-tile granularity (first matmul at ~10us).
- phase 1: RoPE for block t emitted during block t+1 (gap-filling);
  V-transpose staging copies on the idle scalar engine.
- phase 2: 2-tile score groups ([128,1024] PSUM, one ACTIVATE each),
  software-pipelined one group deep; per-head normalize deferred one
  head. PSUM: sps 2x2 + cps 2 + dps/bps 2.
- denominators: one DVE pair-add + one ones-matmul per group.
- phase 3: 2 PSUM banks, 16-matmul accumulation chains ordered so the
  last head's (h=3) contributions come last; gathered halves prefetch
  to SBUF as single 1MB DMAs; bf16 output.
"""

import os
import sys

sys.path.insert(0, "/opt/trn_rl_repo")
import numpy as np


B, S, HID = 2, 2048, 2048
NH, NKV, D = 16, 4, 128
N_CORES = 8
GROUPS = [[0, 1, 2, 3], [4, 5, 6, 7]]
HLOC = 4          # q heads per core
TB = 512          # token block (matmul moving dim)
NTB = S // TB     # 4
HT = HID // 128   # 16 hid tiles
SCALE = float(D) ** -0.5
HS = S // 2       # token half for collectives

LAST_RESULTS = None  # stash for test harness timing


def _analyze_mask(mask):
    """Per (qblock, ktile): live pairs and mixed-mask tiles (deduped)."""
    maskb = np.asarray(mask).astype(bool)
    live = []
    mixd = {}
    uniq = []
    keys = {}
    for qb in range(NTB):
        lv = []
        for kt in range(S // 128):
            sub = maskb[qb * TB:(qb + 1) * TB, kt * 128:(kt + 1) * 128]
            if not sub.any():
                continue
            lv.append(kt)
            if sub.all():
                mixd[(qb, kt)] = None
            else:
                tile = np.ascontiguousarray(sub.T.astype(np.float32))
                kb = tile.tobytes()
                if kb not in keys:
                    keys[kb] = len(uniq)
                    uniq.append(tile)
                mixd[(qb, kt)] = keys[kb]
        live.append(lv)
    return live, mixd, uniq


def _build_program(live, mixd, n_u):
    import concourse.bass as bass  # noqa: F401
    import concourse.mybir as mybir
    from concourse import bacc, tile

    f32 = mybir.dt.float32
    bf16 = mybir.dt.bfloat16
    EXP = mybir.ActivationFunctionType.Exp

    nc = bacc.Bacc("TRN2", target_bir_lowering=False, debug=False,
                   num_devices=N_CORES)

    xT = nc.dram_tensor("xT", [HID, S], bf16, kind="ExternalInput")
    wq = nc.dram_tensor("wq", [HID, HLOC * D], bf16, kind="ExternalInput")
    wk = nc.dram_tensor("wk", [HID, D], bf16, kind="ExternalInput")
    wv = nc.dram_tensor("wv", [HID, D], bf16, kind="ExternalInput")
    wo = nc.dram_tensor("wo", [HID, TB], bf16, kind="ExternalInput")
    cosE = nc.dram_tensor("cosE", [D, S], bf16, kind="ExternalInput")
    sinP = nc.dram_tensor("sinP", [D, S], bf16, kind="ExternalInput")
    pswap = nc.dram_tensor("pswap", [128, 128], bf16, kind="ExternalInput")
    ident = nc.dram_tensor("ident", [128, 128], bf16, kind="ExternalInput")
    ones_in = nc.dram_tensor("ones_in", [128, 1], bf16, kind="ExternalInput")
    onesk1_in = nc.dram_tensor("onesk1_in", [1, 128], bf16, kind="ExternalInput")
    mmask = nc.dram_tensor("mmask", [max(n_u, 1) * 128, TB], bf16,
                           kind="ExternalInput")
    out_o = nc.dram_tensor("o", [S, TB], bf16, kind="ExternalOutput")

    mm = nc.tensor.matmul

    def chunks(lst, n):
        return [lst[i:i + n] for i in range(0, len(lst), n)]

    with tile.TileContext(nc, num_cores=N_CORES) as tc:
        stk0 = nc.allow_low_precision("bf16 kernel; fp32 PSUM accumulate")
        stk0.__enter__()
        with (
            tc.tile_pool(name="const", bufs=1) as cpool,
            tc.tile_pool(name="acts", bufs=1) as apool,
            tc.tile_pool(name="gs", bufs=4) as gpool,
            tc.tile_pool(name="dram", bufs=1, space="DRAM") as dram,
        ):
            qT_s = apool.tile([128, HLOC * S], bf16, tag="qT")
            kT_s = apool.tile([128, S], bf16, tag="kT")
            v_s = apool.tile([128, S], bf16, tag="v")
            ctxT_s = apool.tile([128, HLOC * S], bf16, tag="ctxT")
            wo_s = apool.tile([128, HT * TB], bf16, tag="wo")

            bounce = [dram.tile([128, S], bf16, tag=f"bn{h}",
                                name=f"bounce{h}") for h in range(HLOC - 1)]
            gath = [dram.tile([HLOC * 128, S], bf16, tag=f"g{h}",
                              name=f"gath{h}") for h in range(HLOC - 1)]
            bounce3 = [dram.tile([128, HS], bf16, tag=f"bn3{hf}",
                                 name=f"bounce3{hf}") for hf in range(2)]
            gath3 = [dram.tile([HLOC * 128, HS], bf16, tag=f"g3{hf}",
                               name=f"gath3{hf}") for hf in range(2)]

            # ---------- Phase 1: QKV projections + RoPE + V transpose ----------
            with (
                tc.tile_pool(name="w1", bufs=1) as wpool,
                tc.tile_pool(name="xs", bufs=8) as xpool,
                tc.tile_pool(name="p1", bufs=1, space="PSUM") as p1,
                tc.tile_pool(name="p1b", bufs=2, space="PSUM") as p1b,
                tc.tile_pool(name="st1", bufs=2) as stage,
            ):
                wq_s = wpool.tile([128, HT * HLOC * D], bf16, tag="wq")
                wk_s = wpool.tile([128, HT * D], bf16, tag="wk")
                wv_s = wpool.tile([128, HT * D], bf16, tag="wv")
                ps_s = wpool.tile([128, 128], bf16, tag="ps")
                id_s = wpool.tile([128, 128], bf16, tag="id")
                cos_s = wpool.tile([D, S], bf16, tag="cos")
                sin_s = wpool.tile([D, S], bf16, tag="sin")

                def load_wq(g):
                    # scalar-engine DMA queue: doesn't block x tiles on Sync
                    nc.scalar.dma_start(
                        out=wq_s[:, g * 4 * HLOC * D:(g + 1) * 4 * HLOC * D]
                        .rearrange("p (h n) -> p h n", n=HLOC * D),
                        in_=wq[g * 512:(g + 1) * 512, :]
                        .rearrange("(h p) n -> p h n", p=128))

                load_wq(0)
                nc.scalar.dma_start(
                    out=wk_s[:].rearrange("p (h n) -> p h n", n=D),
                    in_=wk[:].rearrange("(h p) n -> p h n", p=128))
                nc.scalar.dma_start(
                    out=wv_s[:].rearrange("p (h n) -> p h n", n=D),
                    in_=wv[:].rearrange("(h p) n -> p h n", p=128))

                def rope_block(t):
                    cks = [qT_s[:, i * S + t * TB: i * S + (t + 1) * TB]
                           for i in range(HLOC)]
                    cks.append(kT_s[:, t * TB:(t + 1) * TB])
                    for ch in cks:
                        sw = p1b.tile([128, TB], f32, tag="aux", name="swps")
                        mm(sw[:], ps_s[:], ch, start=True, stop=True)
                        swm = stage.tile([128, TB], bf16, tag="swm")
                        nc.vector.tensor_mul(swm[:], sw[:],
                                             sin_s[:, t * TB:(t + 1) * TB])
                        nc.vector.tensor_mul(ch, ch,
                                             cos_s[:, t * TB:(t + 1) * TB])
                        nc.vector.tensor_add(ch, ch, swm[:])

                for t in range(NTB):
                    qps = [p1.tile([128, TB], f32, tag=f"qps{i}", name=f"qps{i}")
                           for i in range(HLOC)]
                    kps = p1.tile([128, TB], f32, tag="kps")
                    vps = p1.tile([128, TB], f32, tag="vps")
                    for h in range(HT):
                        xt = xpool.tile([128, TB], bf16, tag="xt")
                        nc.sync.dma_start(
                            out=xt[:],
                            in_=xT[h * 128:(h + 1) * 128, t * TB:(t + 1) * TB],
                        )
                        if t == 0 and h in (0, 4, 8):
                            load_wq(h // 4 + 1)
                        if t == 0 and h == 7:
                            nc.scalar.dma_start(out=cos_s[:], in_=cosE[:])
                            nc.scalar.dma_start(out=sin_s[:], in_=sinP[:])
                            nc.scalar.dma_start(out=ps_s[:], in_=pswap[:])
                            nc.scalar.dma_start(out=id_s[:], in_=ident[:])
                        st, sp = (h == 0), (h == HT - 1)
                        for i in range(HLOC):
                            mm(qps[i][:],
                               wq_s[:, h * HLOC * D + i * D: h * HLOC * D + (i + 1) * D],
                               xt[:], start=st, stop=sp)
                        mm(kps[:], wk_s[:, h * D:(h + 1) * D], xt[:], start=st, stop=sp)
                        mm(vps[:], wv_s[:, h * D:(h + 1) * D], xt[:], start=st, stop=sp)

                    # Q/K: copy to SBUF on DVE (frees accumulators)
                    for i in range(HLOC):
                        nc.vector.tensor_copy(
                            qT_s[:, i * S + t * TB: i * S + (t + 1) * TB],
                            qps[i][:])
                    nc.vector.tensor_copy(kT_s[:, t * TB:(t + 1) * TB], kps[:])

                    # V chain on the idle scalar engine: stage + transpose
                    vstg = stage.tile([128, TB], bf16, tag="vstg")
                    nc.scalar.copy(vstg[:], vps[:])
                    for i in range(TB // 128):
                        tps = p1b.tile([128, 128], bf16, tag="aux", name="tps")
                        nc.tensor.transpose(tps[:], vstg[:, i * 128:(i + 1) * 128], id_s[:])
                        tt = t * (TB // 128) + i
                        nc.scalar.copy(v_s[:, tt * 128:(tt + 1) * 128], tps[:])

                    if t > 0:
                        rope_block(t - 1)
                rope_block(NTB - 1)

            # ---------- Phase 2: attention, head-major, pipelined ----------
            gsa = [None] * (HLOC - 1)
            gsa3 = [None, None]
            with (
                tc.tile_pool(name="ex", bufs=6) as epool,
                tc.tile_pool(name="bc", bufs=2) as bcpool,
                tc.tile_pool(name="rc", bufs=2) as rcpool,
                tc.tile_pool(name="p2s", bufs=2, space="PSUM") as p2s,
                tc.tile_pool(name="p2c", bufs=2, space="PSUM") as p2c,
                tc.tile_pool(name="p2d", bufs=2, space="PSUM") as p2d,
            ):
                ones_s = cpool.tile([128, 1], bf16, tag="ones")
                nc.scalar.dma_start(out=ones_s[:], in_=ones_in[:])
                onesk1 = cpool.tile([1, 128], bf16, tag="onesk1")
                nc.scalar.dma_start(out=onesk1[:], in_=onesk1_in[:])
                mm_s = None
                if n_u:
                    mm_s = cpool.tile([128, n_u * TB], bf16, tag="mm")
                    nc.scalar.dma_start(
                        out=mm_s[:].rearrange("p (u n) -> p u n", n=TB),
                        in_=mmask[:].rearrange("(u p) n -> p u n", p=128),
                    )
                nc.scalar.dma_start(
                    out=wo_s[:].rearrange("p (h n) -> p h n", n=TB),
                    in_=wo[:].rearrange("(h p) n -> p h n", p=128),
                )

                def normalize(qb, h, cps, dps):
                    """Deferred per-(h,qb) softmax-normalize (+ A2A per head)."""
                    rc = rcpool.tile([1, TB], f32, tag="rc")
                    nc.vector.reciprocal_approx_fast(rc[:], dps[:])
                    rcb = rcpool.tile([1, TB], bf16, tag="rcb")
                    nc.vector.tensor_copy(rcb[:], rc[:])
                    bps = p2d.tile([128, TB], f32, tag="dps", name="bps")
                    mm(bps[:], onesk1[:], rcb[:], start=True, stop=True)
                    bcs = bcpool.tile([128, TB], bf16, tag="bcs")
                    nc.vector.tensor_copy(bcs[:], bps[:])
                    nc.vector.tensor_mul(
                        ctxT_s[:, h * S + qb * TB: h * S + (qb + 1) * TB],
                        cps[:], bcs[:])
                    if h < HLOC - 1 and qb == NTB - 1:
                        nc.sync.dma_start(
                            out=bounce[h][:],
                            in_=ctxT_s[:, h * S:(h + 1) * S])
                        nc.gpsimd.collective_compute(
                            "AllGather",
                            mybir.AluOpType.bypass,
                            replica_groups=GROUPS,
                            ins=[bounce[h].opt()],
                            outs=[gath[h].opt()],
                        )
                        g = gpool.tile([128, HLOC * S], bf16, tag="gs")
                        nc.sync.dma_start(
                            out=g[:].rearrange("p (j n) -> p j n", n=S),
                            in_=gath[h][:].rearrange("(j p) n -> p j n",
                                                     p=128),
                        )
                        gsa[h] = g
                    elif h == HLOC - 1 and qb in (1, 3):
                        hf = qb // 2
                        nc.sync.dma_start(
                            out=bounce3[hf][:],
                            in_=ctxT_s[:, h * S + hf * HS: h * S + (hf + 1) * HS])
                        nc.gpsimd.collective_compute(
                            "AllGather",
                            mybir.AluOpType.bypass,
                            replica_groups=GROUPS,
                            ins=[bounce3[hf].opt()],
                            outs=[gath3[hf].opt()],
                        )
                        g = gpool.tile([128, HLOC * HS], bf16, tag="gs3")
                        nc.sync.dma_start(
                            out=g[:].rearrange("p (j n) -> p j n", n=HS),
                            in_=gath3[hf][:].rearrange("(j p) n -> p j n",
                                                       p=128),
                        )
                        gsa3[hf] = g

                pending_norm = None
                for h in range(HLOC):
                    for qb in range(NTB):
                        lv = live[qb]
                        grps = chunks(lv, 2)
                        ng = len(grps)
                        qslice = qT_s[:, h * S + qb * TB: h * S + (qb + 1) * TB]
                        cps = p2c.tile([128, TB], f32, tag="cps")
                        dps = p2d.tile([1, TB], f32, tag="dps")
                        pend = None  # (gi, kts, ex)

                        def tail(gi, kts, ex):
                            for j, kt in enumerate(kts):
                                u = mixd[(qb, kt)]
                                if u is not None:
                                    exj = ex[:, j * TB:(j + 1) * TB]
                                    nc.vector.tensor_mul(
                                        exj, exj, mm_s[:, u * TB:(u + 1) * TB])
                            for j, kt in enumerate(kts):
                                mm(cps[:], v_s[:, kt * 128:(kt + 1) * 128],
                                   ex[:, j * TB:(j + 1) * TB],
                                   start=(gi == 0 and j == 0),
                                   stop=(gi == ng - 1 and j == len(kts) - 1))
                            for j in range(len(kts)):
                                mm(dps[:], ones_s[:], ex[:, j * TB:(j + 1) * TB],
                                   start=(gi == 0 and j == 0),
                                   stop=(gi == ng - 1 and j == len(kts) - 1))

                        for gi, kts in enumerate(grps):
                            w = TB * len(kts)
                            sps = p2s.tile([128, w], f32, tag="sps")
                            for j, kt in enumerate(kts):
                                mm(sps[:, j * TB:(j + 1) * TB],
                                   kT_s[:, kt * 128:(kt + 1) * 128], qslice,
                                   start=True, stop=True)
                            ex = epool.tile([128, w], bf16, tag="ex")
                            nc.scalar.activation(ex[:], sps[:], EXP, scale=SCALE)
                            if gi == 1 and pending_norm is not None:
                                normalize(*pending_norm)
                                pending_norm = None
                            if pend is not None:
                                tail(*pend)
                            pend = (gi, kts, ex)
                        tail(*pend)
                        if pending_norm is not None:
                            normalize(*pending_norm)
                        pending_norm = (qb, h, cps, dps)
                normalize(*pending_norm)

            # ---------- Phase 3: output projection (column shard) ----------
            with (
                tc.tile_pool(name="ob", bufs=3) as opool,
                tc.tile_pool(name="p3", bufs=4, space="PSUM") as p3,
            ):
                for tt in range(16):
                    ops = p3.tile([128, TB], f32, tag="ops")
                    # h-major so the last-gathered head (h=3) is last
                    for h in range(HLOC):
                        for j in range(HLOC):
                            g = 4 * j + h
                            if h < HLOC - 1:
                                stat = gsa[h][:, j * S + tt * 128:
                                              j * S + (tt + 1) * 128]
                            else:
                                hf, ti = tt // 8, tt % 8
                                stat = gsa3[hf][:, j * HS + ti * 128:
                                                j * HS + (ti + 1) * 128]
                            mm(ops[:], stat,
                               wo_s[:, g * TB:(g + 1) * TB],
                               start=(h == 0 and j == 0),
                               stop=(h == HLOC - 1 and j == HLOC - 1))
                    osb = opool.tile([128, TB], bf16, tag="osb")
                    nc.vector.tensor_copy(osb[:], ops[:])
                    nc.sync.dma_start(out=out_o[tt * 128:(tt + 1) * 128, :],
                                      in_=osb[:])
        stk0.__exit__(None, None, None)
    nc.compile()
    return nc


def kernel(x, wq, wk, wv, wo, cos, sin, mask):
    global LAST_RESULTS
    import ml_dtypes
    from concourse.bass_utils import run_bass_kernel_spmd

    bfnp = ml_dtypes.bfloat16
    x = np.asarray(x, np.float32)
    wq = np.asarray(wq, np.float32)
    wk = np.asarray(wk, np.float32)
    wv = np.asarray(wv, np.float32)
    wo = np.asarray(wo, np.float32)
    cos = np.asarray(cos, np.float32)
    sin = np.asarray(sin, np.float32)

    live, mixd, uniq = _analyze_mask(mask)
    n_u = len(uniq)
    mmask = (np.concatenate(uniq, axis=0) if n_u
             else np.zeros((128, TB), np.float32))

    cosE = np.repeat(cos, 2, axis=1).T
    sp = np.repeat(sin, 2, axis=1).copy()
    sp[:, 0::2] *= -1.0
    sinP = sp.T
    pswap = np.zeros((128, 128), np.float32)
    pswap[np.arange(128), np.arange(128) ^ 1] = 1.0
    ident = np.eye(128, dtype=np.float32)

    nc = _build_program(live, mixd, n_u)

    def b(a):
        return np.ascontiguousarray(np.asarray(a).astype(bfnp))

    in_maps = []
    for c in range(N_CORES):
        bb, j = c // 4, c % 4
        in_maps.append({
            "xT": b(x[bb].T),
            "wq": b(wq[:, 512 * j:512 * (j + 1)]),
            "wk": b(wk[:, 128 * j:128 * (j + 1)]),
            "wv": b(wv[:, 128 * j:128 * (j + 1)]),
            "wo": b(wo[:, 512 * j:512 * (j + 1)]),
            "cosE": b(cosE), "sinP": b(sinP), "pswap": b(pswap),
            "ident": b(ident),
            "ones_in": b(np.ones((128, 1), np.float32)),
            "onesk1_in": b(np.ones((1, 128), np.float32)),
            "mmask": b(mmask),
        })

    res = run_bass_kernel_spmd(nc, in_maps, list(range(N_CORES)))
    LAST_RESULTS = res

    out = np.empty((B, S, HID), np.float32)
    for c in range(N_CORES):
        bb, j = c // 4, c % 4
        out[bb, :, 512 * j:512 * (j + 1)] = np.asarray(
            res.results[c]["o"]).astype(np.float32)
    return out
